# revision 25
# baseline (speedup 1.0000x reference)
"""Multi-head attention block (B=4, S=2048, D=1024, H=16) on 8 TRN2 cores.

Sharding: data-parallel over batch (4 batches x 2 cores) and tensor-parallel
over heads (8 heads per core).  Each core computes, for its (batch, head-group):
  Q^T/K^T (head-dim-major) and V (seq-major) projections, causal attention
  (scores transposed: S^T[k,q] = K Q^T, exp without max-subtraction, row-sum
  via an appended ones-column in the PV matmul), context, and a partial output
  projection with its w_o column slice.  The host sums the two partial outputs
  per batch (the "all-reduce after w_o") and adds b_o.

Matmuls run in bf16 by default (1 PE cycle/row vs fp32's 4); accumulation is
always fp32 in PSUM.  kernel(**inputs) takes full unsharded inputs and returns
the full output.
"""

import numpy as np

import concourse.bass as bass
import concourse.mybir as mybir
import concourse.tile as tile
from concourse import bacc
from concourse.bass_utils import run_bass_kernel_spmd
from concourse.masks import make_identity

B, S, D, H = 4, 2048, 1024, 16
DK = D // H            # 64 head dim
P = 128                # partitions
NCORES = 8
HPC = H // 2           # 8 heads per core
DPC = HPC * DK         # 512 projected dims per core
NPAIR = DPC // P       # 4 head-pairs per core
KT = D // P            # 8 contraction tiles for projections
SC_W = 512             # phase-1 seq chunk width
N_SC = S // SC_W
QC_W = 512             # phase-2 query chunk width
N_QC = S // QC_W
NKB = S // P           # 16 key blocks
F32 = mybir.dt.float32
BF16 = mybir.dt.bfloat16
F32R = mybir.dt.float32r

_NC_CACHE: dict = {}


def _build_nc(causal: bool, reps: int = 1, mmdt: str = "bf16", phases: int = 3) -> bass.Bass:
    """mmdt: 'bf16' (default, 1 cyc/row), 'f32r' (1 cyc/row, flaky on HW),
    or 'f32' (4 cyc/row, exact)."""
    mm_dt = {"bf16": BF16, "f32r": F32R, "f32": F32}[mmdt]
    # bf16: host ships pre-converted bf16 inputs -> DMA directly, no converts.
    # f32r: DMA f32 then round on DVE (verifier requires a rounding producer).
    in_dt = BF16 if mmdt == "bf16" else F32
    needs_cvt = mmdt == "f32r" 

    def mm(out, lhsT, rhs, **kw):
        if mmdt == "f32r":
            lhsT = lhsT.bitcast(F32R)
            rhs = rhs.bitcast(F32R)
        nc.tensor.matmul(out, lhsT=lhsT, rhs=rhs, **kw)

    nc = bacc.Bacc(
        "TRN2",
        debug=False,
        enable_asserts=False,
        target_bir_lowering=False,
        num_devices=NCORES,
    )

    qT = nc.dram_tensor("qT", [D, S], in_dt, kind="ExternalInput").ap()
    kT = nc.dram_tensor("kT", [D, S], in_dt, kind="ExternalInput").ap()
    vT = nc.dram_tensor("vT", [D, S], in_dt, kind="ExternalInput").ap()
    wqT = nc.dram_tensor("wqT", [D, DPC], in_dt, kind="ExternalInput").ap()
    wkT = nc.dram_tensor("wkT", [D, DPC], in_dt, kind="ExternalInput").ap()
    wvT = nc.dram_tensor("wvT", [D, DPC], in_dt, kind="ExternalInput").ap()
    woT = nc.dram_tensor("woT", [DPC, D], in_dt, kind="ExternalInput").ap()
    bq = nc.dram_tensor("bq", [DPC], F32, kind="ExternalInput").ap()
    bk = nc.dram_tensor("bk", [DPC], F32, kind="ExternalInput").ap()
    bv = nc.dram_tensor("bv", [DPC], F32, kind="ExternalInput").ap()
    out = nc.dram_tensor("out", [S, D], F32, kind="ExternalOutput").ap()

    from contextlib import ExitStack

    with tile.TileContext(nc) as tc, ExitStack() as octx:
        if reps > 1:
            octx.enter_context(tc.For_i(0, reps, 1))
        ctx = octx.enter_context(ExitStack())
        singles = ctx.enter_context(tc.tile_pool(name="singles", bufs=1))

        identity = singles.tile([P, P], mm_dt if mmdt == "bf16" else F32)
        make_identity(nc, identity)

        if causal:
            # mask_ext[k, u] = 1.0 if (u - EXT) >= k else 0.0; slicing a QC_W
            # window at offset EXT - c*P gives the causal mask for the c-th
            # diagonal k-block of a query chunk (c = kb - j*NQB).
            EXT = (QC_W // P - 1) * P
            mask_ext = singles.tile([P, QC_W + EXT], mm_dt if mmdt == "bf16" else F32)
            nc.gpsimd.memset(mask_ext, 1.0)
            nc.gpsimd.affine_select(
                out=mask_ext,
                in_=mask_ext,
                compare_op=mybir.AluOpType.is_ge,
                fill=0.0,
                base=-EXT,
                channel_multiplier=-1,
                pattern=[[1, QC_W + EXT]],
            )

        bq_sb = singles.tile([P, NPAIR], F32)
        nc.sync.dma_start(bq_sb, bq.rearrange("(pair p) -> p pair", p=P))
        bk_sb = singles.tile([P, NPAIR], F32)
        nc.sync.dma_start(bk_sb, bk.rearrange("(pair p) -> p pair", p=P))
        bv_sb = singles.tile([P, NPAIR], F32)
        nc.sync.dma_start(bv_sb, bv.rearrange("(pair p) -> p pair", p=P))

        woT_sb = singles.tile([P, NPAIR, D], mm_dt)
        if needs_cvt:
            with tc.tile_pool(name="woraw", bufs=1) as worp:
                woT_raw = worp.tile([P, NPAIR, D], F32, name="woT_raw")
                nc.sync.dma_start(
                    woT_raw, woT.rearrange("(pair p) dm -> p pair dm", p=P))
                nc.vector.tensor_copy(out=woT_sb, in_=woT_raw)
        else:
            nc.sync.dma_start(woT_sb, woT.rearrange("(pair p) dm -> p pair dm", p=P))

        # Persistent activations
        QT_all = singles.tile([P, NPAIR, S], mm_dt)   # [d%128, pair, s]
        KT_all = singles.tile([P, NPAIR, S], mm_dt)
        V_all = singles.tile([P, NKB, HPC, DK + 1], mm_dt)  # [s%128, kb, h, d|1]
        ctxT_all = singles.tile([P, NPAIR, S], mm_dt)
        nc.vector.memset(V_all[:, :, :, DK:DK + 1], 1.0)

        # ---- Phase 1: projections ----
        with (
            tc.tile_pool(name="wpool", bufs=1) as wpool,
            tc.tile_pool(name="xraw", bufs=2) as xraw,
            tc.tile_pool(name="xstage", bufs=4) as xst,
            tc.tile_pool(name="ppsum", bufs=4, space="PSUM") as pp,
        ):
            w_sbs = []
            for which, w_dram in enumerate([wqT, wkT, wvT]):
                w_sb = wpool.tile([P, KT, DPC], mm_dt, name=f"w_sb{which}")
                if needs_cvt:
                    w_raw = xraw.tile([P, KT, DPC], F32, name=f"w_raw{which}")
                    nc.sync.dma_start(
                        w_raw, w_dram.rearrange("(kt p) d -> p kt d", p=P))
                    nc.vector.tensor_copy(out=w_sb, in_=w_raw)
                else:
                    nc.sync.dma_start(
                        w_sb, w_dram.rearrange("(kt p) d -> p kt d", p=P))
                w_sbs.append(w_sb)
            for which, (x_dram, bias_sb) in enumerate(
                [(qT, bq_sb), (kT, bk_sb), (vT, None)]
            ):
                w_sb = w_sbs[which]
                xr = x_dram.rearrange("(kt p) s -> p kt s", p=P)
                for sc in range(N_SC):
                    if needs_cvt:
                        x_raw = xraw.tile([P, KT, SC_W], F32, name="x_raw")
                        nc.sync.dma_start(x_raw, xr[:, :, sc * SC_W:(sc + 1) * SC_W])
                        x_sb = xst.tile([P, KT, SC_W], mm_dt, name="x_sb")
                        nc.vector.tensor_copy(out=x_sb, in_=x_raw)
                    else:
                        x_sb = xst.tile([P, KT, SC_W], in_dt, name="x_sb")
                        nc.sync.dma_start(x_sb, xr[:, :, sc * SC_W:(sc + 1) * SC_W])
                    if which < 2:  # Q, K -> head-major [d, s]
                        dest_all = QT_all if which == 0 else KT_all
                        for pair in range(NPAIR):
                            ps = pp.tile([P, SC_W], F32, name="ps_qk")
                            for kt in range(KT):
                                mm(
                                    ps,
                                    w_sb[:, kt, pair * P:(pair + 1) * P],
                                    x_sb[:, kt, :],
                                    start=(kt == 0),
                                    stop=(kt == KT - 1),
                                )
                            nc.vector.tensor_scalar_add(
                                out=dest_all[:, pair, sc * SC_W:(sc + 1) * SC_W],
                                in0=ps,
                                scalar1=bias_sb[:, pair:pair + 1],
                            )
                    else:  # V -> seq-major [s, d] (bias deferred to ctx^T)
                        for ss in range(SC_W // P):
                            ps = pp.tile([P, DPC], F32, name="ps_v")
                            for kt in range(KT):
                                mm(
                                    ps,
                                    x_sb[:, kt, ss * P:(ss + 1) * P],
                                    w_sb[:, kt, :],
                                    start=(kt == 0),
                                    stop=(kt == KT - 1),
                                )
                            sblk = sc * (SC_W // P) + ss
                            nc.vector.tensor_copy(
                                out=V_all[:, sblk, :, 0:DK],
                                in_=ps.rearrange("p (h d) -> p h d", h=HPC),
                            )

        # ---- Phase 2: attention ----
        NQB = QC_W // P   # 128-row query sub-blocks per chunk
        if phases < 2:
            return nc
        with (
            tc.tile_pool(name="ptpool", bufs=30) as ptp,
            tc.tile_pool(name="stage", bufs=2) as stg,
            tc.tile_pool(name="little", bufs=8) as lit,
            tc.tile_pool(name="spsum", bufs=3, space="PSUM") as sp,
            tc.tile_pool(name="cpsum", bufs=3, space="PSUM") as cp,
            tc.tile_pool(name="tpsum", bufs=2, space="PSUM") as tp,
        ):
            for pair in range(NPAIR):
                ctx_stage = stg.tile([P, NKB, P], mm_dt if mmdt == "bf16" else F32, name="ctx_stage")
                for hp in range(2):
                    h = pair * 2 + hp
                    psl = slice(hp * DK, (hp + 1) * DK)

                    def emit_st(j):
                        qcols = slice(j * QC_W, (j + 1) * QC_W)
                        kb_hi = min(NKB, (j + 1) * NQB) if causal else NKB
                        pt_tiles = {}
                        for kb in range(kb_hi):
                            ps = sp.tile([P, QC_W], F32, name="ps_s")
                            mm(
                                ps,
                                KT_all[psl, pair, kb * P:(kb + 1) * P],
                                QT_all[psl, pair, qcols],
                                start=True,
                                stop=True,
                            )
                            ptt = ptp.tile([P, QC_W], mm_dt, name="pt")
                            nc.scalar.activation(
                                ptt, ps, mybir.ActivationFunctionType.Exp,
                                scale=1.0 / np.sqrt(DK),
                            )
                            if causal and kb >= j * NQB:
                                c = kb - j * NQB
                                off = EXT - c * P
                                nc.vector.tensor_mul(
                                    ptt, ptt, mask_ext[:, off:off + QC_W]
                                )
                            pt_tiles[kb] = ptt
                        return pt_tiles

                    def emit_pv(j, pt_tiles):
                        for qq in range(NQB):
                            qb = j * NQB + qq
                            kmax = (qb + 1) if causal else NKB
                            cps = cp.tile([P, DK + 1], F32, name="cps")
                            for kb in range(kmax):
                                nc.tensor.matmul(
                                    cps,
                                    lhsT=pt_tiles[kb][:, qq * P:(qq + 1) * P],
                                    rhs=V_all[:, kb, h, :],
                                    start=(kb == 0),
                                    stop=(kb == kmax - 1),
                                )
                            recip = lit.tile([P, 1], F32, name="recip")
                            nc.vector.reciprocal(recip, cps[:, DK:DK + 1])
                            nc.vector.tensor_scalar_mul(
                                ctx_stage[:, qb, psl], cps[:, 0:DK], scalar1=recip
                            )

                    # software pipeline: scores for chunk j+1 are emitted
                    # before chunk j's PV so the exp (ACT) latency is hidden
                    prev = None
                    for j in range(N_QC):
                        cur = emit_st(j)
                        if prev is not None:
                            emit_pv(j - 1, prev)
                        prev = cur
                    emit_pv(N_QC - 1, prev)
                # transpose ctx to head-major and add v-bias
                for sb in range(NKB):
                    tps = tp.tile([P, P], mm_dt if mmdt == "bf16" else F32, name="tps")
                    nc.tensor.transpose(tps, ctx_stage[:, sb, :], identity)
                    nc.vector.tensor_scalar_add(
                        out=ctxT_all[:, pair, sb * P:(sb + 1) * P],
                        in0=tps,
                        scalar1=bv_sb[:, pair:pair + 1],
                    )

        # ---- Phase 3: output projection (partial; host sums core pairs) ----
        if phases < 3:
            return nc
        with (
            tc.tile_pool(name="opsum", bufs=4, space="PSUM") as op,
            tc.tile_pool(name="ostage", bufs=3) as ost,
        ):
            NDC = D // 512
            for sb in range(NKB):
                for dmc in range(NDC):
                    ps = op.tile([P, 512], F32, name="ps_o")
                    for pair in range(NPAIR):
                        mm(
                            ps,
                            ctxT_all[:, pair, sb * P:(sb + 1) * P],
                            woT_sb[:, pair, dmc * 512:(dmc + 1) * 512],
                            start=(pair == 0),
                            stop=(pair == NPAIR - 1),
                        )
                    o_sb = ost.tile([P, 512], F32, name="o_sb")
                    nc.vector.tensor_copy(out=o_sb, in_=ps)
                    nc.sync.dma_start(
                        out[sb * P:(sb + 1) * P, dmc * 512:(dmc + 1) * 512], o_sb
                    )

    if not nc.is_finalized():
        nc.finalize()
    return nc


def _get_nc(causal: bool, reps: int = 1, **kw) -> bass.Bass:
    key = (causal, reps, tuple(sorted(kw.items())))
    if key not in _NC_CACHE:
        _NC_CACHE[key] = _build_nc(causal, reps, **kw)
    return _NC_CACHE[key]


def _make_in_maps(q, k, v, w_q, w_k, w_v, w_o, b_q, b_k, b_v, in_np=None):
    import ml_dtypes
    if in_np is None:
        in_np = ml_dtypes.bfloat16
    in_maps = []
    qb = [np.ascontiguousarray(q[b].T.astype(in_np)) for b in range(B)]
    kb = [np.ascontiguousarray(k[b].T.astype(in_np)) for b in range(B)]
    vb = [np.ascontiguousarray(v[b].T.astype(in_np)) for b in range(B)]
    for c in range(NCORES):
        b, g = divmod(c, 2)
        hsl = slice(g * DPC, (g + 1) * DPC)
        in_maps.append({
            "qT": qb[b],
            "kT": kb[b],
            "vT": vb[b],
            "wqT": np.ascontiguousarray(w_q[hsl, :].T.astype(in_np)),
            "wkT": np.ascontiguousarray(w_k[hsl, :].T.astype(in_np)),
            "wvT": np.ascontiguousarray(w_v[hsl, :].T.astype(in_np)),
            "woT": np.ascontiguousarray(w_o[:, hsl].T.astype(in_np)),
            "bq": np.ascontiguousarray(b_q[hsl]),
            "bk": np.ascontiguousarray(b_k[hsl]),
            "bv": np.ascontiguousarray(b_v[hsl]),
        })
    return in_maps


def kernel(q, k, v, mask, w_q, b_q, w_k, b_k, w_v, b_v, w_o, b_o, **run_kwargs):
    q = np.asarray(q, np.float32)
    k = np.asarray(k, np.float32)
    v = np.asarray(v, np.float32)
    w_q = np.asarray(w_q, np.float32)
    w_k = np.asarray(w_k, np.float32)
    w_v = np.asarray(w_v, np.float32)
    w_o = np.asarray(w_o, np.float32)
    b_q = np.asarray(b_q, np.float32)
    b_k = np.asarray(b_k, np.float32)
    b_v = np.asarray(b_v, np.float32)
    b_o = np.asarray(b_o, np.float32)

    mask_b = np.asarray(mask).reshape(S, S).astype(bool)
    causal = bool(np.array_equal(mask_b, np.tril(np.ones((S, S), bool))))
    if not causal:
        assert mask_b.all(), "only causal or all-ones masks are supported"

    nc = _get_nc(causal)
    in_maps = _make_in_maps(q, k, v, w_q, w_k, w_v, w_o, b_q, b_k, b_v)

    res = run_bass_kernel_spmd(nc, in_maps, core_ids=list(range(NCORES)), **run_kwargs)
    outs = [r["out"] for r in res.results]
    full = np.stack(
        [outs[2 * b] + outs[2 * b + 1] + b_o[None, :] for b in range(B)]
    ).astype(np.float32)
    kernel.last_result = res
    return full


kernel.last_result = None
